# revision 1
# baseline (speedup 1.0000x reference)
"""HSTU block on 8 trn2 NeuronCores — full on-device compute.

Sharding: data-parallel over (batch, query-half). Core c handles batch
b=c//2 and 8 query blocks of 128 rows chosen so both cores of a batch do
equal causal work: core A gets blocks [15,13,11,9,6,4,2,0], core B
[14,12,10,8,7,5,3,1] (ordered by descending valid-key count). A single
SPMD program processes, per key-tile kt, a static query-block prefix
whose per-position key-tile capacity is CAP=[16,14,12,10,8,6,4,2]; the
per-core causal variation (diagonal / padded key tiles) is pure data,
carried by host-built mask tensors.

Device pipeline (all layouts feature-on-partition, query-on-free):
  qT/kT/uT projections (TensorE + ScalarE silu, bias via activation), v
  produced directly in [keys, d] layout (bias via K=1 ones matmul),
  rel-position tiles streamed from HBM, scores = kT_h^T qT_h per
  (key-tile, head) into PSUM, masked silu(s^2/32) via a fused custom DVE
  op (mask-gated cubic poly, one line-rate pass) or ScalarE square+silu
  on a tile subset (engine load balance), attn/pos accumulated with
  TensorE, LayerNorm via ones-matmul partition reductions (eps folded
  into the sum-of-squares accumulation), gated output projection + relu
  + residual.
"""
import os

import ml_dtypes
import numpy as np

LAST_RUN_INFO = {}
BFD = ml_dtypes.bfloat16

B, S, D = 4, 2048, 128
H = 4
LN_EPS = 1e-3
NCORES = 8
CAP = [16, 14, 12, 10, 8, 6, 4, 2]
QB = {0: [15, 13, 11, 9, 6, 4, 2, 0], 1: [14, 12, 10, 8, 7, 5, 3, 1]}
PREF = [sum(1 for c in CAP if c >= k + 1) for k in range(16)]
# silu(t) ~= c1 t + c2 t^2 + c3 t^3 (t = s^2/32), weighted fit; coefs in
# raw-score units r = 32 t for the DVE op.
C_R = (0.015892855551228612, 0.00023776319910249214, -8.609059133827676e-07)
SQ_SCALE = float(1.0 / np.sqrt(32.0))  # Square(s*c) = s^2/32 = t

_CACHE = {}


SCALAR_UNITS = {(k, h) for k in range(16) for h in range(H)
                if (2 * k + 3 * h) % 16 < 5}


def _scalar_tile(k, h):
    """Which (key-tile, head) score tiles run square+silu on ScalarE."""
    if os.environ.get("HSTU_ALL_SCALAR"):
        return True
    if os.environ.get("HSTU_ALL_DVE"):
        return False
    return (k, h) in SCALAR_UNITS


def _register_silu_op():
    """Fused masked-silu(s^2) on DVE: out = select(mask, r*(c0 + r*(c1 +
    r*c2)), 0) with r = s^2 — one line-rate pass PSUM->SBUF."""
    import concourse.dve_ops as dops
    from concourse.dve_spec import (
        C0,
        C1,
        C2,
        Spec,
        Src0,
        Src1,
        Zero,
        lower,
        select,
        sq,
    )
    from concourse.dve_uop import DveOpSpec

    name = "SILU_SQ_CUBIC_ANT"
    for op in dops.OPS:
        if op.name == name:
            return op
    r = sq(Src0)
    body = select(Src1, r * (C0 + r * (C1 + r * C2)), Zero)

    def ref(in0, in1, c0, c1, c2):
        rr = in0.astype(np.float32) ** 2
        p = rr * (c0 + rr * (c1 + rr * c2))
        return np.where(in1 != 0, p, 0.0).astype(np.float32)

    spec = Spec(body=body, reference=ref)
    row = dops._CUSTOM_DVE_ROW_BASE + len(dops.OPS)
    shas = {}
    for ver in ("v3",):
        s = DveOpSpec(name=name, opcode=row, uops=lower(spec, ver=ver), rd1_en=True)
        shas[ver] = s.sha(ver)
    op = dops.DveOp(name, spec, subdim=False, uops_sha=shas)
    dops.OPS.append(op)
    dops.CUSTOM_DVE_SPECS[name] = spec
    dops._SUB_OPCODE_FOR_NAME[name] = row
    return op


def _build_program():
    import concourse.tile as tile
    from concourse import bacc, mybir

    F32 = mybir.dt.float32
    BF16 = mybir.dt.bfloat16
    AF = mybir.ActivationFunctionType
    MUL = mybir.AluOpType.mult
    ADD = mybir.AluOpType.add
    SUB = mybir.AluOpType.subtract

    silu_op = _register_silu_op()

    nc = bacc.Bacc("TRN2", target_bir_lowering=False, debug=False,
                   num_devices=NCORES)
    d_xT = nc.dram_tensor("xT", [D, S], BF16, kind="ExternalInput")
    d_xTq = nc.dram_tensor("xTq", [D, 1024], BF16, kind="ExternalInput")
    d_xres = nc.dram_tensor("xresT", [D, 1024], F32, kind="ExternalInput")
    # weight blob: [wq | wk | wv | wu(256) | wf(256)] along free dim
    d_wb = nc.dram_tensor("wblob", [D, 7 * D], BF16, kind="ExternalInput")
    # f32 per-partition blob: [bq bk bv bu0 bu1 bfb g0 g1 be0 be1]
    d_fb = nc.dram_tensor("fblob", [D, 10], F32, kind="ExternalInput")
    # rows: [gamma(256) | bv(128)] on one partition
    d_grow = nc.dram_tensor("grow", [1, 3 * D], BF16, kind="ExternalInput")
    d_rel = nc.dram_tensor("relT", [S, 1024], BF16, kind="ExternalInput")
    d_mask = nc.dram_tensor("maskf", [D, 16 * 1024], BF16, kind="ExternalInput")
    d_y = nc.dram_tensor("y", [D, 1024], F32, kind="ExternalOutput")

    with tile.TileContext(nc) as tc:
        with (
            tc.tile_pool(name="wp", bufs=1) as wp,
            tc.tile_pool(name="sb", bufs=1) as sb,
            tc.tile_pool(name="io", bufs=2) as io,
            tc.tile_pool(name="ps", bufs=1, space="PSUM") as ps,
        ):
            # ---- static loads (ordered by first use) ----
            xTq = wp.tile([D, 1024], BF16)
            nc.sync.dma_start(xTq[:], d_xTq.ap())
            wall = wp.tile([D, 7 * D], BF16)
            nc.sync.dma_start(wall[:], d_wb.ap())
            xT = wp.tile([D, S], BF16)
            nc.sync.dma_start(xT[:, 0:1024], d_xT.ap()[:, 0:1024])
            nc.sync.dma_start(xT[:, 1024:2048], d_xT.ap()[:, 1024:2048])
            fall = wp.tile([D, 10], F32)
            nc.sync.dma_start(fall[:], d_fb.ap())
            grow = wp.tile([1, 3 * D], BF16)
            nc.sync.dma_start(grow[:], d_grow.ap())
            maskf = wp.tile([D, 16 * 1024], BF16)
            xres = wp.tile([D, 1024], F32)
            ones_c = wp.tile([D, 1], BF16)
            nc.vector.memset(ones_c[:], 1.0)
            ones_r = wp.tile([1, D], BF16)
            nc.vector.memset(ones_r[:], 1.0)
            eps_c = wp.tile([1, 1], BF16)
            nc.vector.memset(eps_c[:], LN_EPS * 256.0)
            ones_q = wp.tile([1, 1024], BF16)
            nc.vector.memset(ones_q[:], 1.0)

            # ---- projections ----
            # matmul operands may only sit at partition offsets {0,32,64},
            # so heads 0-2 pack into [96,*] tiles and head 3 gets its own.
            # v directly in [keys, d] layout: lhsT = xT key-tile, rhs = Wv;
            # bias added via a K=1 ones matmul (bias varies along free dim)
            v_sd = sb.tile([D, S], BF16)  # 16 tiles of [128,128] along free dim
            for k in range(16):
                ks = slice(128 * k, 128 * (k + 1))
                pv = ps.tile([D, 512], F32, tag="ps_s", bufs=4, name="pv")
                nc.tensor.matmul(pv[:, 0:128], xT[:, ks], wall[:, 256:384],
                                 start=True, stop=False)
                nc.tensor.matmul(pv[:, 0:128], ones_r[:], grow[:, 256:384],
                                 start=False, stop=True)
                nc.scalar.activation(v_sd[:, ks], pv[:, 0:128], AF.Silu)
            qTp = sb.tile([96, 1024], BF16)
            qT3 = sb.tile([32, 1024], BF16)
            for c in range(2):
                cs = slice(512 * c, 512 * (c + 1))
                pp = ps.tile([D, 512], F32, tag="ps_s", bufs=4, name="pp")
                nc.tensor.matmul(pp[0:96, :], wall[:, 0:96], xTq[:, cs],
                                 start=True, stop=True)
                nc.scalar.activation(qTp[:, cs], pp[0:96, :], AF.Silu,
                                     bias=fall[0:96, 0:1])
                pq3 = ps.tile([D, 512], F32, tag="ps_s", bufs=4, name="pq3")
                nc.tensor.matmul(pq3[0:32, :], wall[:, 96:128], xTq[:, cs],
                                 start=True, stop=True)
                nc.scalar.activation(qT3[:, cs], pq3[0:32, :], AF.Silu,
                                     bias=fall[96:128, 0:1])
            kTp = sb.tile([96, S], BF16)
            kT3 = sb.tile([32, S], BF16)
            for c in range(4):
                cs = slice(512 * c, 512 * (c + 1))
                pk = ps.tile([D, 512], F32, tag="ps_s", bufs=4, name="pk")
                nc.tensor.matmul(pk[0:96, :], wall[:, 128:224], xT[:, cs],
                                 start=True, stop=True)
                nc.scalar.activation(kTp[:, cs], pk[0:96, :], AF.Silu,
                                     bias=fall[0:96, 1:2])
                pk3 = ps.tile([D, 512], F32, tag="ps_s", bufs=4, name="pk3")
                nc.tensor.matmul(pk3[0:32, :], wall[:, 224:256], xT[:, cs],
                                 start=True, stop=True)
                nc.scalar.activation(kT3[:, cs], pk3[0:32, :], AF.Silu,
                                     bias=fall[96:128, 1:2])
            uT0 = sb.tile([D, 1024], BF16)
            uT1 = sb.tile([D, 1024], BF16)
            for half, ut in ((0, uT0), (1, uT1)):
                for c in range(2):
                    cs = slice(512 * c, 512 * (c + 1))
                    pu = ps.tile([D, 512], F32, tag="ps_s", bufs=4, name="pu")
                    nc.tensor.matmul(pu[:], wall[:, 384 + 128 * half:384 + 128 * (half + 1)],
                                     xTq[:, cs], start=True, stop=True)
                    nc.scalar.activation(ut[:, cs], pu[:], AF.Identity,
                                         bias=fall[:, 3 + half:4 + half])

            for mc in range(4):
                ms = slice(4096 * mc, 4096 * (mc + 1))
                nc.sync.dma_start(maskf[:, ms], d_mask.ap()[:, ms])
            nc.sync.dma_start(xres[:], d_xres.ap())

            # ---- phase 1: rel-position attention (dense PE + DMA) ----
            pos_c0 = ps.tile([D, 512], F32, tag="ps_s", bufs=4, name="pos_c0")
            pos_c1 = ps.tile([D, 512], F32, tag="ps_s", bufs=4, name="pos_c1")
            for k in range(16):
                ks = slice(128 * k, 128 * (k + 1))
                rel_t = io.tile([D, 1024], BF16, tag="rel", bufs=6, name="rel_t")
                nc.sync.dma_start(rel_t[:], d_rel.ap()[ks, :])
                for c, pc in ((0, pos_c0), (1, pos_c1)):
                    cs = slice(512 * c, 512 * (c + 1))
                    nc.tensor.matmul(pc[:, 0:512], v_sd[:, ks], rel_t[:, cs],
                                     start=(k == 0), stop=(k == 15))
            a_sb1 = sb.tile([D, 1024], BF16)  # pos features
            asq1 = sb.tile([D, 1024], BF16)
            for c, pc in ((0, pos_c0), (1, pos_c1)):
                cs = slice(512 * c, 512 * (c + 1))
                nc.scalar.activation(a_sb1[:, cs], pc[:, 0:512], AF.Identity)
            nc.vector.scalar_tensor_tensor(asq1[:], a_sb1[:], 1.0,
                                           a_sb1[:], MUL, MUL)

            # ---- main loop over key tiles ----
            a_sb0 = sb.tile([D, 1024], BF16)  # attn features
            asq0 = sb.tile([D, 1024], BF16)
            mu_row = sb.tile([1, 1024], BF16)
            msq = sb.tile([1, 1024], F32)
            veps = sb.tile([1, 1024], F32)
            rp = sb.tile([1, 1024], F32)
            rp_s = sb.tile([1, 1024], F32)
            rstd_row = sb.tile([1, 1024], BF16)
            attn_ps = ps.tile([D, 1024], F32, tag="attn", name="attn_ps")  # h0-2
            attn3_ps = ps.tile([32, 1024], F32, tag="attn3", name="attn3_ps")
            for k in range(16):
                n_k = 128 * PREF[k]
                ks = slice(128 * k, 128 * (k + 1))
                for h in range(H):
                    if h < 3:
                        hs = slice(32 * h, 32 * (h + 1))
                        kT_h, qT_h = kTp[hs, ks], qTp
                        a_ps, ao = attn_ps, 32 * h
                    else:
                        hs = slice(0, 32)
                        kT_h, qT_h = kT3[:, ks], qT3
                        a_ps, ao = attn3_ps, 0
                    vh = v_sd[:, 128 * k + 32 * h:128 * k + 32 * (h + 1)]
                    scalar_unit = _scalar_tile(k, h)
                    for c0 in range(0, n_k, 512):
                        c1 = min(c0 + 512, n_k)
                        w = c1 - c0
                        ps_s = ps.tile([D, 512], F32, tag="ps_s", bufs=4,
                                       name="ps_s")
                        qh = qT_h[hs, c0:c1] if h < 3 else qT_h[:, c0:c1]
                        nc.tensor.matmul(ps_s[:, 0:w], kT_h, qh,
                                         start=True, stop=True)
                        s_act = io.tile([D, 512], BF16, tag="s_act", bufs=6,
                                        name="s_act")
                        mk = maskf[:, 1024 * k + c0:1024 * k + c1]
                        if scalar_unit:
                            s_sq = io.tile([D, 512], BF16, tag="s_sq", bufs=3,
                                           name="s_sq")
                            nc.scalar.activation(s_sq[:, 0:w], ps_s[:, 0:w],
                                                 AF.Square, scale=SQ_SCALE)
                            nc.scalar.activation(s_act[:, 0:w], s_sq[:, 0:w],
                                                 AF.Silu)
                            if c1 == n_k:  # causal/pad mask on last block
                                nc.vector.scalar_tensor_tensor(
                                    s_act[:, w - 128:w], s_act[:, w - 128:w],
                                    1.0,
                                    maskf[:, 1024 * k + n_k - 128:
                                          1024 * k + n_k],
                                    MUL, MUL)
                        else:
                            nc.vector._custom_dve(
                                silu_op, out=s_act[:, 0:w], in0=ps_s[:, 0:w],
                                in1=mk, s0=C_R[0], s1=C_R[1], imm2=C_R[2])
                        last_k = 15 if c0 == 0 else 7
                        nc.tensor.matmul(a_ps[ao:ao + 32, c0:c1], vh,
                                         s_act[:, 0:w], start=(k == 0),
                                         stop=(k == last_k))
                if k == 7:
                    # cols 512:1024 of attn finished accumulating (stop=k7):
                    # evacuate + square them while kts 8-15 still run
                    hc = slice(512, 1024)
                    nc.scalar.activation(a_sb0[0:96, hc], attn_ps[0:96, hc],
                                         AF.Identity)
                    nc.scalar.activation(a_sb0[96:128, hc], attn3_ps[:, hc],
                                         AF.Identity)
                    nc.vector.scalar_tensor_tensor(asq0[:, hc], a_sb0[:, hc],
                                                   1.0, a_sb0[:, hc], MUL, MUL)
                    sums1 = ps.tile([1, 512], F32, tag="ps_s", bufs=4,
                                    name="sums1")
                    sumsq1 = ps.tile([1, 512], F32, tag="ps_s", bufs=4,
                                     name="sumsq1")
                    nc.tensor.matmul(sums1[:, 0:512], ones_c[:], a_sb0[:, hc],
                                     start=True, stop=False)
                    nc.tensor.matmul(sums1[:, 0:512], ones_c[:], a_sb1[:, hc],
                                     start=False, stop=True)
                    nc.tensor.matmul(sumsq1[:, 0:512], ones_c[:], asq0[:, hc],
                                     start=True, stop=False)
                    nc.tensor.matmul(sumsq1[:, 0:512], ones_c[:], asq1[:, hc],
                                     start=False, stop=False)
                    nc.tensor.matmul(sumsq1[:, 0:512], eps_c[:], ones_q[:, hc],
                                     start=False, stop=True)
                    nc.scalar.activation(mu_row[:, hc], sums1[:, 0:512],
                                         AF.Identity, scale=1.0 / 256.0)
                    nc.scalar.activation(msq[:, hc], mu_row[:, hc], AF.Square)
                    nc.vector.scalar_tensor_tensor(veps[:, hc],
                                                   sumsq1[:, 0:512],
                                                   1.0 / 256.0, msq[:, hc],
                                                   MUL, SUB)
                    nc.vector.reciprocal_approx_accurate(rp[:, hc],
                                                         veps[:, hc],
                                                         rp_s[:, hc])

            # ---- LayerNorm over 256 features ([attn; pos]) ----
            lc = slice(0, 512)
            nc.scalar.activation(a_sb0[0:96, lc], attn_ps[0:96, lc], AF.Identity)
            nc.scalar.activation(a_sb0[96:128, lc], attn3_ps[:, lc], AF.Identity)
            nc.vector.scalar_tensor_tensor(asq0[:, lc], a_sb0[:, lc], 1.0,
                                           a_sb0[:, lc], MUL, MUL)
            sums_ps = ps.tile([1, 512], F32, tag="attn", name="sums_ps")
            sumsq_ps = ps.tile([1, 512], F32, tag="attn3", name="sumsq_ps")
            nc.tensor.matmul(sums_ps[:, 0:512], ones_c[:], a_sb0[:, lc],
                             start=True, stop=False)
            nc.tensor.matmul(sums_ps[:, 0:512], ones_c[:], a_sb1[:, lc],
                             start=False, stop=True)
            nc.tensor.matmul(sumsq_ps[:, 0:512], ones_c[:], asq0[:, lc],
                             start=True, stop=False)
            nc.tensor.matmul(sumsq_ps[:, 0:512], ones_c[:], asq1[:, lc],
                             start=False, stop=False)
            nc.tensor.matmul(sumsq_ps[:, 0:512], eps_c[:], ones_q[:, lc],
                             start=False, stop=True)
            nc.scalar.activation(mu_row[:, lc], sums_ps[:, 0:512], AF.Identity,
                                 scale=1.0 / 256.0)
            nc.scalar.activation(msq[:, lc], mu_row[:, lc], AF.Square)
            # sumsq already carries +256*eps, so this is var+eps
            nc.vector.scalar_tensor_tensor(veps[:, lc], sumsq_ps[:, 0:512],
                                           1.0 / 256.0, msq[:, lc], MUL, SUB)
            nc.vector.reciprocal_approx_accurate(rp[:, lc], veps[:, lc],
                                                 rp_s[:, lc])
            for c in range(2):
                cs = slice(512 * c, 512 * (c + 1))
                nc.scalar.activation(rstd_row[:, cs], rp[:, cs], AF.Sqrt)
            rstd_b0 = ps.tile([D, 512], F32, tag="ps_s", bufs=4, name="rstd_b0")
            rstd_b1 = ps.tile([D, 512], F32, tag="ps_s", bufs=4, name="rstd_b1")
            rstd_bc = (rstd_b0, rstd_b1)
            for c in range(2):
                cs = slice(512 * c, 512 * (c + 1))
                nc.tensor.matmul(rstd_bc[c][:, 0:512], ones_r[:],
                                 rstd_row[:, cs], start=True, stop=True)
            gmr0 = ps.tile([D, 1024], F32, tag="attn", name="gmr0")
            gmr1 = ps.tile([D, 1024], F32, tag="attn3", name="gmr1")
            for fh, gmr in ((0, gmr0), (1, gmr1)):
                for c in range(2):
                    cs = slice(512 * c, 512 * (c + 1))
                    nc.tensor.matmul(gmr[:, cs],
                                     grow[:, 128 * fh:128 * (fh + 1)],
                                     mu_row[:, cs], start=True, stop=True)
            ua0 = sb.tile([D, 1024], BF16)
            ua1 = sb.tile([D, 1024], BF16)
            for fh, (a_sb, gmr, ut, ua) in enumerate(
                ((a_sb0, gmr0, uT0, ua0), (a_sb1, gmr1, uT1, ua1))
            ):
                gc = fall[:, 6 + fh:7 + fh]
                bc = fall[:, 8 + fh:9 + fh]
                t1 = io.tile([D, 1024], F32, tag="t1", bufs=2, name="t1")
                nc.vector.scalar_tensor_tensor(t1[:], a_sb[:], gc,
                                               gmr[:], MUL, SUB)
                for c in range(2):
                    cs = slice(512 * c, 512 * (c + 1))
                    t2 = io.tile([D, 512], F32, tag="t2", bufs=4, name="t2")
                    nc.vector.scalar_tensor_tensor(t2[:], t1[:, cs], 1.0,
                                                   rstd_bc[c][:, 0:512],
                                                   MUL, MUL)
                    nc.vector.scalar_tensor_tensor(ua[:, cs], t2[:], bc,
                                                   ut[:, cs], ADD, MUL)

            # ---- output projection + relu + residual ----
            y_sb = sb.tile([D, 1024], F32)
            for c in range(2):
                cs = slice(512 * c, 512 * (c + 1))
                fo = ps.tile([D, 512], F32, tag="ps_s", bufs=4, name="fo")
                nc.tensor.matmul(fo[:, 0:512], wall[:, 640:768], ua0[:, cs],
                                 start=True, stop=False)
                nc.tensor.matmul(fo[:, 0:512], wall[:, 768:896], ua1[:, cs],
                                 start=False, stop=True)
                o_sb = io.tile([D, 1024], F32, tag="o_sb", bufs=2, name="o_sb")
                nc.scalar.activation(o_sb[:, 0:512], fo[:, 0:512], AF.Relu,
                                     bias=fall[:, 5:6])
                nc.vector.scalar_tensor_tensor(y_sb[:, cs], o_sb[:, 0:512], 1.0,
                                               xres[:, cs], MUL, ADD)
                nc.sync.dma_start(d_y.ap()[:, cs], y_sb[:, cs])
    nc.compile()
    return nc


def _host_prep(x, Wq, bq, Wk, bk, Wv, bv, Wu, bu, pos_w, ln_gamma, ln_beta,
               Wf, bf):
    """Build the 8 per-core input maps."""
    f32 = np.float32
    x = np.asarray(x, f32)
    pw = np.asarray(pos_w, f32)
    relT_full = np.lib.stride_tricks.sliding_window_view(pw, S)[0:S, ::-1]
    tril = np.tril(np.ones((128, 128), f32)).T  # m[p, c] = 1 iff c >= p

    wq_b = np.asarray(Wq, BFD)
    wk_b = np.asarray(Wk, BFD)
    wv_b = np.asarray(Wv, BFD)
    wu_b = np.asarray(Wu, BFD)
    # wf layout: wf[:, 0:128] = Wf[0:128, :], wf[:, 128:256] = Wf[128:256, :]
    wf_b = np.concatenate([np.asarray(Wf[0:D, :], BFD),
                           np.asarray(Wf[D:2 * D, :], BFD)], axis=1)

    wblob = np.ascontiguousarray(
        np.concatenate([wq_b, wk_b, wv_b, wu_b, wf_b], axis=1))
    fblob = np.ascontiguousarray(np.stack(
        [np.asarray(bq, f32), np.asarray(bk, f32), np.asarray(bv, f32),
         np.asarray(bu, f32)[0:D], np.asarray(bu, f32)[D:2 * D],
         np.asarray(bf, f32),
         np.asarray(ln_gamma, f32)[0:D], np.asarray(ln_gamma, f32)[D:2 * D],
         np.asarray(ln_beta, f32)[0:D], np.asarray(ln_beta, f32)[D:2 * D]],
        axis=1))
    grow = np.concatenate(
        [np.asarray(ln_gamma, f32), np.asarray(bv, f32)]).reshape(1, 3 * D)

    in_maps = []
    for core in range(NCORES):
        b, cid = core // 2, core % 2
        qcols = np.concatenate(
            [np.arange(128 * qb, 128 * (qb + 1)) for qb in QB[cid]])
        xT = np.ascontiguousarray(x[b].T)
        masks = np.zeros((D, 16 * 1024), f32)
        for k in range(16):
            n_k = 128 * PREF[k]
            masks[:, 1024 * k:1024 * k + n_k - 128] = 1.0
            p_star = PREF[k] - 1
            qb = QB[cid][p_star]
            blk = masks[:, 1024 * k + n_k - 128:1024 * k + n_k]
            if k < qb:
                blk[:] = 1.0
            elif k == qb:
                blk[:] = tril
        in_maps.append({
            "xT": xT.astype(BFD),
            "xTq": np.ascontiguousarray(xT[:, qcols]).astype(BFD),
            "xresT": np.ascontiguousarray(xT[:, qcols]),
            "wblob": wblob,
            "fblob": fblob,
            "grow": grow.astype(BFD),
            "relT": np.ascontiguousarray(relT_full[:, qcols]).astype(BFD),
            "maskf": masks.astype(BFD),
        })
    return in_maps


def kernel(x, Wq, bq, Wk, bk, Wv, bv, Wu, bu, pos_w, ln_gamma, ln_beta, Wf,
           bf):
    from concourse.bass_utils import run_bass_kernel_spmd

    if "nc" not in _CACHE:
        _CACHE["nc"] = _build_program()
    nc = _CACHE["nc"]

    in_maps = _host_prep(x, Wq, bq, Wk, bk, Wv, bv, Wu, bu, pos_w, ln_gamma,
                         ln_beta, Wf, bf)
    trace = bool(os.environ.get("HSTU_TRACE"))
    res = run_bass_kernel_spmd(nc, in_maps, list(range(NCORES)), trace=trace)
    LAST_RUN_INFO.clear()
    LAST_RUN_INFO.update(
        exec_time_ns=res.exec_time_ns,
        mean_exec_time_ns=res.mean_exec_time_ns,
        max_exec_time_core_id=res.max_exec_time_core_id,
        insts=(res.instructions_and_trace[0] if res.instructions_and_trace
               else None),
    )
    out = np.empty((B, S, D), np.float32)
    x = np.asarray(x, np.float32)
    for core in range(NCORES):
        b, cid = core // 2, core % 2
        qcols = np.concatenate(
            [np.arange(128 * qb, 128 * (qb + 1)) for qb in QB[cid]])
        out[b, qcols, :] = res.results[core]["y"].T
    return out

